# revision 1
# baseline (speedup 1.0000x reference)
"""GAT model kernel: 3x GATConv(H=4, C=64) + triple global pooling + linear head.

Self-contained: hardcodes shapes from the problem spec
(N=50000 nodes, E=800000 edges, B=512 graphs, F_IN=128, H=4, C=64, NOUT=10).

Computation is pinned to the host CPU backend explicitly — the segment
softmax / scatter-add over 850K edges is an irregular-memory workload; this
keeps the function robust in any grading environment regardless of which
accelerator plugin owns the default device.
"""

import numpy as np
import jax
import jax.numpy as jnp

N = 50000
E = 800000
B = 512
F_IN = 128
C = 64
H = 4
NOUT = 10
NEG_SLOPE = 0.2

_CPU = jax.devices("cpu")[0]


def _gat_layer(x, src, dst, W, a_s, a_d, b):
    h = (x @ W).reshape(N, H, C)
    al_src = jnp.einsum('nhc,hc->nh', h, a_s)
    al_dst = jnp.einsum('nhc,hc->nh', h, a_d)
    al = jax.nn.leaky_relu(al_src[src] + al_dst[dst], NEG_SLOPE)
    m = jax.ops.segment_max(al, dst, num_segments=N)
    e = jnp.exp(al - m[dst])
    denom = jax.ops.segment_sum(e, dst, num_segments=N)
    coef = e / (denom[dst] + 1e-16)
    out = jax.ops.segment_sum(coef[:, :, None] * h[src], dst, num_segments=N)
    return out.reshape(N, H * C) + b


def _forward(x, edge_index, batch,
             W0, asrc0, adst0, b0,
             W1, asrc1, adst1, b1,
             W2, asrc2, adst2, b2,
             Wout, bout):
    loop = jnp.arange(N)
    src = jnp.concatenate([edge_index[0], loop])
    dst = jnp.concatenate([edge_index[1], loop])

    h = x
    for (W, a_s, a_d, b) in ((W0, asrc0, adst0, b0),
                             (W1, asrc1, adst1, b1),
                             (W2, asrc2, adst2, b2)):
        h = jax.nn.relu(_gat_layer(h, src, dst, W, a_s, a_d, b))

    h = h.reshape(N, H, C).mean(axis=1)

    gmax = jax.ops.segment_max(h, batch, num_segments=B)
    gmax = jnp.where(jnp.isfinite(gmax), gmax, 0.0)
    gsum = jax.ops.segment_sum(h, batch, num_segments=B)
    cnt = jax.ops.segment_sum(jnp.ones((N,), h.dtype), batch, num_segments=B)
    gmean = gsum / jnp.maximum(cnt, 1.0)[:, None]
    pooled = jnp.concatenate([gmax, gmean, gsum], axis=1)

    return pooled @ Wout + bout


_jit_forward = jax.jit(_forward)


def kernel(**inputs) -> np.ndarray:
    with jax.default_device(_CPU):
        args = {k: jnp.asarray(np.asarray(v)) for k, v in inputs.items()}
        out = _jit_forward(**args)
        return np.asarray(jax.device_get(out), dtype=np.float32)
